# revision 1
# baseline (speedup 1.0000x reference)
"""3-layer GAT (8 heads x 64 ch) + global mean pool + FC + log_softmax on 8 Trainium2 cores.

Sharding: nodes (and their incoming edges) are partitioned across the 8 cores;
GAT weights are replicated; per layer each core computes h=x@W for its node
shard, the [h | a_src] rows are AllGathered into a replicated table, and each
core gathers source rows for its edges with indirect DMA (one row per
partition per op — the HW limit).

Edge aggregation per 128-node dst tile is hybrid:
- "round" columns: round r holds each node's r-th incoming edge in that
  node's own partition, so dst == partition. a_dst adds directly, padding is
  a -1e30 bias before exp, and the weighted scatter-add is an
  identity-stationary matmul accumulating into PSUM.
- "overflow" columns: edges beyond the per-tile round cap go through a
  one-hot matmul path (is_equal against iota, PE transpose for the per-edge
  a_dst gather).
Numerator and softmax denominator accumulate in the same PSUM group; one
divide per tile. The next layer's x@W (stage A) is fused into the per-tile
finalize so PE work hides under the gather-DMA-bound phase. Graph mean-pool
partials are AllReduced at the end.
"""

import numpy as np

import concourse.bass as bass
import concourse.mybir as mybir
import concourse.tile as tile
from concourse import bacc
from concourse.bass_utils import run_bass_kernel_spmd
from concourse.masks import make_identity

# problem constants (hardcoded per contract)
N, E, F_IN, H, C, G, NCLS = 50000, 400000, 128, 8, 64, 64, 10
HC = H * C  # 512
NEG = 0.2
EPS = 1e-16

NCORES = 8
P = 128
NSH = N // NCORES          # 6250 nodes per core
NT = (NSH + P - 1) // P    # 49 dst tiles per core
NSHP = NT * P              # 6272 padded rows per core
NFULL = NCORES * NSHP      # 50176 rows in the gathered table
ROWW = HC + H              # 520: h | a_src
WEXT = HC + 2 * H          # 528: W | Wa_src | Wa_dst
PAD = 999.0                # one-hot miss marker for padded overflow slots
MASKNEG = -1.0e30          # pre-exp bias masking padded round slots
# AllGather split boundaries (tile granularity) for pipelining behind compute
SPLIT_TILES = (0, 49)
SPLIT_ROWS = tuple(t * P for t in SPLIT_TILES)

F32 = mybir.dt.float32
I32 = mybir.dt.int32


def _preprocess_edges(edge_index):
    """Assign edges (incl. self loops) to the dst-owning core; build per-tile
    round columns (node's r-th edge in its own partition) plus overflow
    one-hot chunk columns.

    Returns (plan, esrcT, edstT):
      plan: list of (R_t, C_t) per tile — shared by all cores.
      esrcT[k] int32 [P, TOTCOL]: gather row ids (pad -> 0).
      edstT[k] f32 [P, TOTCOL]: round cols -> 0.0 real / MASKNEG pad;
                                chunk cols -> dst slot or PAD."""
    src = np.concatenate([edge_index[0], np.arange(N, dtype=np.int64)])
    dst = np.concatenate([edge_index[1], np.arange(N, dtype=np.int64)])
    core = dst // NSH
    dloc = dst - core * NSH
    tile_of = dloc // P
    slot = dloc - tile_of * P
    # gather table layout: SPLITS segments, each rank-major over its row range
    sk = src // NSH
    sr = src % NSH
    split_rows = np.asarray(SPLIT_ROWS, np.int64)
    seg = np.searchsorted(split_rows[1:], sr, side="right")
    r0 = split_rows[seg]
    r1 = split_rows[seg + 1]
    gid = NCORES * r0 + sk * (r1 - r0) + (sr - r0)

    deg = np.zeros((NCORES, NT, P), np.int64)
    np.add.at(deg, (core, tile_of, slot), 1)
    maxdeg_t = deg.max(axis=(0, 2))  # [NT]

    # choose the round cap per tile: gathers dominate, one-hot chunk columns
    # carry ~25% extra compute
    R_ts = np.zeros(NT, np.int64)
    C_ts = np.zeros(NT, np.int64)
    for t in range(NT):
        best = None
        for rcap in range(1, int(maxdeg_t[t]) + 1):
            r = min(int(maxdeg_t[t]), rcap)
            ovf = np.maximum(deg[:, t, :] - rcap, 0).sum(axis=1)
            c = int(np.ceil(ovf / P).max())
            cost = r + 1.25 * c
            if best is None or cost < best[0]:
                best = (cost, r, c)
        _, R_ts[t], C_ts[t] = best
    plan = [(int(R_ts[t]), int(C_ts[t])) for t in range(NT)]
    colbase = np.zeros(NT, np.int64)
    colbase[1:] = np.cumsum(R_ts + C_ts)[:-1]
    TOTCOL = int((R_ts + C_ts).sum())

    esrcT = np.zeros((NCORES, P, TOTCOL), np.int32)
    edstT = np.empty((NCORES, P, TOTCOL), np.float32)
    for k in range(NCORES):
        # default fill: rounds masked, chunks PAD
        for t in range(NT):
            b = colbase[t]
            edstT[k, :, b:b + C_ts[t]] = PAD
            edstT[k, :, b + C_ts[t]:b + C_ts[t] + R_ts[t]] = MASKNEG
        m = core == k
        t_k, s_k, g_k = tile_of[m], slot[m], gid[m]
        order = np.argsort(t_k * P + s_k, kind="stable")
        t_k, s_k, g_k = t_k[order], s_k[order], g_k[order]
        node = t_k * P + s_k
        start = np.zeros(NT * P + 1, np.int64)
        np.add.at(start[1:], node, 1)
        start = np.cumsum(start)
        j = np.arange(len(node)) - start[node]  # rank within node
        rmax = R_ts[t_k]
        isr = j < rmax
        # round entries (after the C_t chunk columns)
        rcol = colbase[t_k[isr]] + C_ts[t_k[isr]] + j[isr]
        esrcT[k, s_k[isr], rcol] = g_k[isr].astype(np.int32)
        edstT[k, s_k[isr], rcol] = 0.0
        # overflow entries: sequential position within each tile
        to, so, go = t_k[~isr], s_k[~isr], g_k[~isr]
        oorder = np.argsort(to, kind="stable")
        to, so, go = to[oorder], so[oorder], go[oorder]
        ostart = np.zeros(NT + 1, np.int64)
        np.add.at(ostart[1:], to, 1)
        ostart = np.cumsum(ostart)
        q = np.arange(len(to)) - ostart[to]
        col = colbase[to] + q // P
        row = q % P
        esrcT[k, row, col] = go.astype(np.int32)
        edstT[k, row, col] = so.astype(np.float32)
    return plan, esrcT, edstT


def _ext_weights(W, a_s, a_d):
    """[K, 528] = [W | W@A_s | W@A_d] so h, a_src, a_dst come from one matmul."""
    K = W.shape[0]
    Wr = W.reshape(K, H, C)
    ws = np.einsum("fhc,hc->fh", Wr, a_s)
    wd = np.einsum("fhc,hc->fh", Wr, a_d)
    Wx = np.concatenate([W, ws, wd], axis=1).astype(np.float32)
    nk = K // P
    return np.ascontiguousarray(Wx.reshape(nk, P, WEXT).transpose(1, 0, 2))


def _build_nc(plan):
    TOTCOL = sum(r + c for r, c in plan)
    nc = bacc.Bacc("TRN2", target_bir_lowering=False, debug=False,
                   num_devices=NCORES)

    x_ext = nc.dram_tensor("x0", [NSHP, F_IN], F32, kind="ExternalInput")
    esrc_ext = nc.dram_tensor("esrc", [P, TOTCOL], I32, kind="ExternalInput")
    edst_ext = nc.dram_tensor("edst", [P, TOTCOL], F32, kind="ExternalInput")
    w1_ext = nc.dram_tensor("w1", [P, 1, WEXT], F32, kind="ExternalInput")
    w2_ext = nc.dram_tensor("w2", [P, 4, WEXT], F32, kind="ExternalInput")
    w3_ext = nc.dram_tensor("w3", [P, 4, WEXT], F32, kind="ExternalInput")
    b1_ext = nc.dram_tensor("b1r", [P, HC], F32, kind="ExternalInput")
    b2_ext = nc.dram_tensor("b2r", [P, HC], F32, kind="ExternalInput")
    b3_ext = nc.dram_tensor("b3r", [P, C], F32, kind="ExternalInput")
    pool_ext = nc.dram_tensor("poolidx", [P, NT], F32, kind="ExternalInput")
    invc_ext = nc.dram_tensor("invcnt", [G, 1], F32, kind="ExternalInput")
    fcw_ext = nc.dram_tensor("fcw", [C, NCLS], F32, kind="ExternalInput")
    fcb_ext = nc.dram_tensor("fcbr", [G, NCLS], F32, kind="ExternalInput")
    out_ext = nc.dram_tensor("out", [G, NCLS], F32, kind="ExternalOutput")

    rg = [list(range(NCORES))]

    with tile.TileContext(nc) as tc:
        with (
            tc.tile_pool(name="const", bufs=1) as cpool,
            tc.tile_pool(name="work", bufs=3) as wpool,
            tc.tile_pool(name="gat", bufs=10) as gpool,
            tc.tile_pool(name="ps", bufs=1, space="PSUM") as pspool,
            tc.tile_pool(name="dram", bufs=1, space="DRAM") as dpool,
        ):
            # ---- constants ----
            iota_i = cpool.tile([P, P], I32)
            nc.gpsimd.iota(iota_i[:], pattern=[[1, P]], base=0, channel_multiplier=0)
            iota_f = cpool.tile([P, P], F32)
            nc.vector.tensor_copy(iota_f[:], iota_i[:])
            ident = cpool.tile([P, P], F32)
            make_identity(nc, ident[:])

            w1_s = cpool.tile([P, 1, WEXT], F32)
            nc.sync.dma_start(out=w1_s[:], in_=w1_ext[:])
            w2_s = cpool.tile([P, 4, WEXT], F32)
            nc.sync.dma_start(out=w2_s[:], in_=w2_ext[:])
            w3_s = cpool.tile([P, 4, WEXT], F32)
            nc.sync.dma_start(out=w3_s[:], in_=w3_ext[:])
            b1_s = cpool.tile([P, HC], F32)
            nc.sync.dma_start(out=b1_s[:], in_=b1_ext[:])
            b2_s = cpool.tile([P, HC], F32)
            nc.sync.dma_start(out=b2_s[:], in_=b2_ext[:])
            b3_s = cpool.tile([P, C], F32)
            nc.sync.dma_start(out=b3_s[:], in_=b3_ext[:])
            pool_s = cpool.tile([P, NT], F32)
            nc.sync.dma_start(out=pool_s[:], in_=pool_ext[:])
            invc_s = cpool.tile([G, 1], F32)
            nc.sync.dma_start(out=invc_s[:], in_=invc_ext[:])
            fcw_s = cpool.tile([C, NCLS], F32)
            nc.sync.dma_start(out=fcw_s[:], in_=fcw_ext[:])
            fcb_s = cpool.tile([G, NCLS], F32)
            nc.sync.dma_start(out=fcb_s[:], in_=fcb_ext[:])
            es_all = cpool.tile([P, TOTCOL], I32)
            nc.sync.dma_start(out=es_all[:], in_=esrc_ext[:])
            ed_all = cpool.tile([P, TOTCOL], F32)
            nc.sync.dma_start(out=ed_all[:], in_=edst_ext[:])
            adst_a = cpool.tile([P, NT * H], F32)
            adst_b = cpool.tile([P, NT * H], F32)

            # ---- DRAM buffers ----
            hx_local = dpool.tile([NSHP, ROWW], F32)
            hx_fulls = [
                dpool.tile([NFULL, ROWW], F32, addr_space="Shared",
                           name=f"hx_full{i}")
                for i in range(3)
            ]
            pool_in = dpool.tile([G, C], F32)
            pool_out = dpool.tile([G, C], F32, addr_space="Shared")

            w_tiles = (w1_s, w2_s, w3_s)
            b_tiles = (b1_s, b2_s, b3_s)
            adst_of = (adst_a, adst_b, adst_a)
            split_end = {SPLIT_TILES[i + 1] - 1: i
                         for i in range(len(SPLIT_TILES) - 1)}

            def emit_split_ag(layer, seg):
                r0, r1 = SPLIT_ROWS[seg], SPLIT_ROWS[seg + 1]
                go = NCORES * r0
                nc.gpsimd.collective_compute(
                    "AllGather", mybir.AluOpType.bypass, replica_groups=rg,
                    ins=[hx_local[r0:r1, :]],
                    outs=[hx_fulls[layer][go:go + NCORES * (r1 - r0), :]],
                )

            def stage_a(xt, layer, t):
                """xt: SBUF [P, K] node-tile features for `layer`; emits
                [h | a_src] -> hx_local rows and a_dst -> adst_of[layer]."""
                K = F_IN if layer == 0 else HC
                nk = K // P
                w_s = w_tiles[layer]
                h_ps = pspool.tile([P, HC], F32, tag="big", bufs=3, name="h_ps")
                a_ps = pspool.tile([P, 2 * H], F32, tag="small", bufs=3,
                                   name="a_ps")
                for j in range(nk):
                    xT_ps = pspool.tile([P, P], F32, tag="trans", bufs=2,
                                        name="xT_ps")
                    nc.tensor.transpose(out=xT_ps[:], in_=xt[:, j * P:(j + 1) * P],
                                        identity=ident[:])
                    xT = wpool.tile([P, P], F32, tag="xT", name="xT")
                    nc.scalar.copy(xT[:], xT_ps[:])
                    nc.tensor.matmul(out=h_ps[:], lhsT=xT[:], rhs=w_s[:, j, 0:HC],
                                     start=(j == 0), stop=(j == nk - 1))
                    nc.tensor.matmul(out=a_ps[:], lhsT=xT[:],
                                     rhs=w_s[:, j, HC:WEXT],
                                     start=(j == 0), stop=(j == nk - 1))
                hx_t = wpool.tile([P, ROWW], F32, tag="hx_t", name="hx_t")
                nc.vector.tensor_copy(hx_t[:, 0:HC // 2], h_ps[:, 0:HC // 2])
                nc.scalar.copy(hx_t[:, HC // 2:HC], h_ps[:, HC // 2:HC])
                nc.vector.tensor_copy(hx_t[:, HC:ROWW], a_ps[:, 0:H])
                nc.vector.tensor_copy(
                    adst_of[layer][:, t * H:(t + 1) * H], a_ps[:, H:2 * H])
                nc.sync.dma_start(out=hx_local[t * P:(t + 1) * P, :], in_=hx_t[:])

            # ---- layer-0 stage A (from input features) ----
            for t in range(NT):
                xt = wpool.tile([P, F_IN], F32, tag="xt0", name="xt")
                nc.sync.dma_start(out=xt[:], in_=x_ext[t * P:(t + 1) * P, :])
                stage_a(xt, 0, t)
                if t in split_end:
                    emit_split_ag(0, split_end[t])

            pool_ps = None
            for layer in range(3):
                hx_full = hx_fulls[layer]
                b_s = b_tiles[layer]
                adst_cur = adst_of[layer]
                if layer == 2:
                    pool_ps = pspool.tile([G, C], F32, tag="small", bufs=3,
                                          name="pool_ps")
                ch0 = 0
                for t in range(NT):
                    R_t, C_t = plan[t]
                    ncol = R_t + C_t
                    num_ps = pspool.tile([P, HC], F32, tag="big", bufs=3,
                                         name="num_ps")
                    den_acc = wpool.tile([P, H], F32, tag="den_acc",
                                         name="den_acc")
                    den_ps = None
                    if C_t > 0:
                        den_ps = pspool.tile([P, H], F32, tag="small", bufs=3,
                                             name="den_ps")
                    a_d = adst_cur[:, t * H:(t + 1) * H]

                    for cc in range(ncol):
                        c = ch0 + cc
                        is_round = cc >= C_t
                        first = cc == 0
                        last = cc == ncol - 1
                        gt = gpool.tile([P, ROWW], F32, tag="gt", name="gt")
                        nc.gpsimd.indirect_dma_start(
                            out=gt[:], out_offset=None,
                            in_=hx_full[:],
                            in_offset=bass.IndirectOffsetOnAxis(
                                ap=es_all[:, c:c + 1], axis=0),
                        )
                        ex = gpool.tile([P, H], F32, tag="ex", name="ex")
                        if is_round:
                            # dst == partition: direct adds, mask via bias
                            logit = gpool.tile([P, H], F32, tag="logit",
                                               name="logit")
                            nc.vector.tensor_add(logit[:], gt[:, HC:ROWW], a_d)
                            nc.vector.tensor_scalar_add(logit[:], logit[:],
                                                        ed_all[:, c:c + 1])
                            lr = gpool.tile([P, H], F32, tag="lr", name="lr")
                            nc.vector.tensor_scalar_mul(lr[:], logit[:], NEG)
                            nc.vector.tensor_tensor(out=lr[:], in0=lr[:],
                                                    in1=logit[:],
                                                    op=mybir.AluOpType.max)
                            nc.scalar.activation(ex[:], lr[:],
                                                 mybir.ActivationFunctionType.Exp)
                            lhs = ident
                        else:
                            oh = gpool.tile([P, P], F32, tag="oh", name="oh")
                            nc.vector.tensor_tensor(
                                out=oh[:],
                                in0=ed_all[:, c:c + 1].to_broadcast([P, P]),
                                in1=iota_f[:], op=mybir.AluOpType.is_equal)
                            ohT_ps = pspool.tile([P, P], F32, tag="trans",
                                                 bufs=2, name="ohT_ps")
                            nc.tensor.transpose(out=ohT_ps[:], in_=oh[:],
                                                identity=ident[:])
                            ohT = gpool.tile([P, P], F32, tag="ohT", name="ohT")
                            nc.scalar.copy(ohT[:], ohT_ps[:])
                            adpe_ps = pspool.tile([P, H], F32, tag="small",
                                                  bufs=3, name="adpe_ps")
                            nc.tensor.matmul(out=adpe_ps[:], lhsT=ohT[:],
                                             rhs=a_d, start=True, stop=True)
                            logit = gpool.tile([P, H], F32, tag="logit",
                                               name="logit")
                            nc.vector.tensor_add(logit[:], gt[:, HC:ROWW],
                                                 adpe_ps[:])
                            lr = gpool.tile([P, H], F32, tag="lr", name="lr")
                            nc.vector.tensor_scalar_mul(lr[:], logit[:], NEG)
                            nc.vector.tensor_tensor(out=lr[:], in0=lr[:],
                                                    in1=logit[:],
                                                    op=mybir.AluOpType.max)
                            nc.scalar.activation(ex[:], lr[:],
                                                 mybir.ActivationFunctionType.Exp)
                            lhs = oh
                        msg = gpool.tile([P, HC], F32, tag="msg", name="msg")
                        for h in range(H):
                            dst_sl = msg[:, h * C:(h + 1) * C]
                            src_sl = gt[:, h * C:(h + 1) * C]
                            if h % 2 == 0:
                                nc.vector.tensor_scalar_mul(dst_sl, src_sl,
                                                            ex[:, h:h + 1])
                            else:
                                nc.scalar.mul(dst_sl, src_sl, ex[:, h:h + 1])
                        nc.tensor.matmul(out=num_ps[:], lhsT=lhs[:], rhs=msg[:],
                                         start=first, stop=last)
                        if is_round:
                            if cc == C_t:
                                nc.vector.tensor_copy(den_acc[:], ex[:])
                            else:
                                nc.vector.tensor_add(den_acc[:], den_acc[:],
                                                     ex[:])
                        else:
                            nc.tensor.matmul(out=den_ps[:], lhsT=lhs[:],
                                             rhs=ex[:], start=(cc == 0),
                                             stop=(cc == C_t - 1))

                    den = wpool.tile([P, H], F32, tag="den", name="den")
                    if den_ps is not None:
                        nc.vector.tensor_add(den[:], den_acc[:], den_ps[:])
                        nc.vector.tensor_scalar_add(den[:], den[:], EPS)
                    else:
                        nc.vector.tensor_scalar_add(den[:], den_acc[:], EPS)
                    rec = wpool.tile([P, H], F32, tag="rec", name="rec")
                    nc.vector.reciprocal(rec[:], den[:])
                    if layer < 2:
                        ot = wpool.tile([P, HC], F32, tag="ot", name="ot")
                        for h in range(H):
                            dst_sl = ot[:, h * C:(h + 1) * C]
                            src_sl = num_ps[:, h * C:(h + 1) * C]
                            if h % 2 == 0:
                                nc.vector.tensor_scalar_mul(dst_sl, src_sl,
                                                            rec[:, h:h + 1])
                            else:
                                nc.scalar.mul(dst_sl, src_sl, rec[:, h:h + 1])
                        ot2 = wpool.tile([P, HC], F32, tag="ot2", name="ot2")
                        nc.vector.tensor_add(ot2[:], ot[:], b_s[:])
                        ot3 = wpool.tile([P, HC], F32, tag="ot3", name="ot3")
                        nc.scalar.activation(ot3[:], ot2[:],
                                             mybir.ActivationFunctionType.Relu)
                        # fused next-layer stage A on this finished tile
                        stage_a(ot3, layer + 1, t)
                        if t in split_end:
                            emit_split_ag(layer + 1, split_end[t])
                    else:
                        hm = wpool.tile([P, C], F32, tag="hm", name="hm")
                        nc.vector.tensor_scalar_mul(hm[:], num_ps[:, 0:C],
                                                    rec[:, 0:1])
                        for h in range(1, H):
                            hmt = wpool.tile([P, C], F32, tag="hmt", name="hmt")
                            if h % 2 == 0:
                                nc.vector.tensor_scalar_mul(
                                    hmt[:], num_ps[:, h * C:(h + 1) * C],
                                    rec[:, h:h + 1])
                            else:
                                nc.scalar.mul(hmt[:], num_ps[:, h * C:(h + 1) * C],
                                              rec[:, h:h + 1])
                            nc.vector.tensor_add(hm[:], hm[:], hmt[:])
                        hs = wpool.tile([P, C], F32, tag="hs", name="hs")
                        nc.vector.tensor_scalar_mul(hs[:], hm[:], 1.0 / H)
                        nc.vector.tensor_add(hs[:], hs[:], b3_s[:])
                        h3 = wpool.tile([P, C], F32, tag="h3", name="h3")
                        nc.scalar.activation(h3[:], hs[:],
                                             mybir.ActivationFunctionType.Relu)
                        poh = wpool.tile([P, G], F32, tag="poh", name="poh")
                        nc.vector.tensor_tensor(
                            out=poh[:], in0=pool_s[:, t:t + 1].to_broadcast([P, G]),
                            in1=iota_f[:, 0:G], op=mybir.AluOpType.is_equal)
                        nc.tensor.matmul(out=pool_ps[:], lhsT=poh[:], rhs=h3[:],
                                         start=(t == 0), stop=(t == NT - 1))
                    ch0 += ncol

            # ---- pool AllReduce + FC + log_softmax ----
            psb = wpool.tile([G, C], F32)
            nc.vector.tensor_copy(psb[:], pool_ps[:])
            nc.sync.dma_start(out=pool_in[:], in_=psb[:])
            nc.gpsimd.collective_compute(
                "AllReduce", mybir.AluOpType.add, replica_groups=rg,
                ins=[pool_in[:]], outs=[pool_out[:]],
            )
            pld = wpool.tile([G, C], F32)
            nc.sync.dma_start(out=pld[:], in_=pool_out[:])
            nc.vector.tensor_scalar_mul(pld[:], pld[:], invc_s[:, 0:1])
            pT_ps = pspool.tile([G, C], F32, tag="small", bufs=3, name="pT_ps")
            nc.tensor.transpose(out=pT_ps[:], in_=pld[:], identity=ident[0:G, 0:C])
            pT = wpool.tile([C, G], F32)
            nc.scalar.copy(pT[:], pT_ps[:])
            z_ps = pspool.tile([G, NCLS], F32, tag="small", bufs=3, name="z_ps")
            nc.tensor.matmul(out=z_ps[:], lhsT=pT[:], rhs=fcw_s[:],
                             start=True, stop=True)
            z = wpool.tile([G, NCLS], F32)
            nc.vector.tensor_add(z[:], z_ps[:], fcb_s[:])
            zm = wpool.tile([G, 1], F32)
            nc.vector.tensor_reduce(zm[:], z[:], axis=mybir.AxisListType.X,
                                    op=mybir.AluOpType.max)
            zs = wpool.tile([G, NCLS], F32)
            nc.vector.tensor_scalar_sub(zs[:], z[:], zm[:, 0:1])
            ze = wpool.tile([G, NCLS], F32)
            nc.scalar.activation(ze[:], zs[:], mybir.ActivationFunctionType.Exp)
            zsum = wpool.tile([G, 1], F32)
            nc.vector.tensor_reduce(zsum[:], ze[:], axis=mybir.AxisListType.X,
                                    op=mybir.AluOpType.add)
            zl = wpool.tile([G, 1], F32)
            nc.scalar.activation(zl[:], zsum[:], mybir.ActivationFunctionType.Ln)
            zo = wpool.tile([G, NCLS], F32)
            nc.vector.tensor_scalar_sub(zo[:], zs[:], zl[:, 0:1])
            nc.sync.dma_start(out=out_ext[:], in_=zo[:])

    nc.compile()
    return nc


def _prepare_inputs(x, edge_index, batch, W1, a1s, a1d, b1, W2, a2s, a2d, b2,
                    W3, a3s, a3d, b3, fcw, fcb):
    plan, esrcT, edstT = _preprocess_edges(np.asarray(edge_index, np.int64))
    x = np.asarray(x, np.float32)
    batch = np.asarray(batch, np.int64)

    shared = {
        "w1": _ext_weights(np.asarray(W1, np.float32), np.asarray(a1s, np.float32),
                           np.asarray(a1d, np.float32)),
        "w2": _ext_weights(np.asarray(W2, np.float32), np.asarray(a2s, np.float32),
                           np.asarray(a2d, np.float32)),
        "w3": _ext_weights(np.asarray(W3, np.float32), np.asarray(a3s, np.float32),
                           np.asarray(a3d, np.float32)),
        "b1r": np.tile(np.asarray(b1, np.float32)[None, :], (P, 1)),
        "b2r": np.tile(np.asarray(b2, np.float32)[None, :], (P, 1)),
        "b3r": np.tile(np.asarray(b3, np.float32)[None, :], (P, 1)),
        "fcw": np.asarray(fcw, np.float32),
        "fcbr": np.tile(np.asarray(fcb, np.float32)[None, :], (G, 1)),
        "invcnt": (1.0 / np.maximum(
            np.bincount(batch, minlength=G), 1.0)).astype(np.float32)[:, None],
    }

    in_maps = []
    for k in range(NCORES):
        xk = np.zeros((NSHP, F_IN), np.float32)
        xk[:NSH] = x[k * NSH:(k + 1) * NSH]
        pidx = np.full((NSHP,), PAD, np.float32)
        pidx[:NSH] = batch[k * NSH:(k + 1) * NSH]
        poolidx = np.ascontiguousarray(pidx.reshape(NT, P).T)  # [P, NT]
        in_maps.append({
            "x0": xk,
            "esrc": np.ascontiguousarray(esrcT[k]),
            "edst": np.ascontiguousarray(edstT[k]),
            "poolidx": poolidx,
            **shared,
        })
    return plan, in_maps


_CACHE = {}


def _get_nc(plan):
    key = tuple(plan)
    if key not in _CACHE:
        _CACHE[key] = _build_nc(plan)
    return _CACHE[key]


def kernel(x, edge_index, batch, W1, a1s, a1d, b1, W2, a2s, a2d, b2,
           W3, a3s, a3d, b3, fcw, fcb, _trace=False, _results=None):
    plan, in_maps = _prepare_inputs(x, edge_index, batch, W1, a1s, a1d, b1,
                                    W2, a2s, a2d, b2, W3, a3s, a3d, b3, fcw, fcb)
    nc = _get_nc(plan)
    res = run_bass_kernel_spmd(nc, in_maps, core_ids=list(range(NCORES)),
                               trace=_trace)
    if _results is not None:
        _results.append(res)
    return res.results[0]["out"]



# revision 10
# speedup vs baseline: 1215.9384x; 1215.9384x over previous
"""3-layer GAT (8 heads x 64 ch) + global mean pool + FC + log_softmax on 8 Trainium2 cores.

v2 redesign of the edge phase around batched per-tile operations:
- Nodes (and incoming edges) partitioned across 8 cores; weights replicated.
- Per layer each core computes h|a_src rows (bf16, channel-major) for its
  node shard; rows are AllGathered (in split segments, pipelined behind the
  tile loop) into a replicated DRAM table.
- Per 128-dst-node tile: ONE wide indirect DMA gathers all edge source rows
  (amortizing SWDGE descriptor-generation cost), attention logits /
  leaky-relu / exp / msg-scale run as a handful of wide strided-AP ops
  (channel-major rows put heads innermost so the alpha broadcast keeps the
  DVE 2x bf16 mode), and the weighted scatter-add is per-column
  identity/one-hot bf16 matmuls accumulating into PSUM.
- finalize fuses divide-by-denominator + bias + relu into one Activation op
  (full-tensor scale/bias APs), and the next layer's x@W stage is fused per
  tile so PE work hides under the gather phase.
- Graph mean-pool partials AllReduced at the end.
"""

import numpy as np
from ml_dtypes import bfloat16 as np_bf16

import concourse.bass as bass
import concourse.mybir as mybir
import concourse.tile as tile
from concourse import bacc
from concourse.bass_utils import run_bass_kernel_spmd
from concourse.masks import make_identity

# problem constants (hardcoded per contract)
N, E, F_IN, H, C, G, NCLS = 50000, 400000, 128, 8, 64, 64, 10
HC = H * C  # 512
NEG = 0.2
EPS = 1e-16

NCORES = 8
P = 128
NSH = N // NCORES          # 6250 nodes per core
NT = (NSH + P - 1) // P    # 49 dst tiles per core
NSHP = NT * P              # 6272 padded rows per core
NFULL = NCORES * NSHP      # 50176 rows in the gathered table
ROWW = HC + H              # 520: h (chan-major) | a_src
WEXT = HC + 2 * H          # 528: W | Wa_src | Wa_dst
PAD = 999.0                # one-hot miss marker for padded overflow slots
MASKNEG = -1.0e30          # pre-exp bias masking padded round slots
CCAP = 4                   # max one-hot chunk columns per tile (PSUM bank)
NCAP = 16                  # max total columns per tile (SBUF budget)
# AllGather split boundaries (tile granularity) for pipelining behind compute
SPLIT_TILES = (0, 49)
SPLIT_ROWS = tuple(t * P for t in SPLIT_TILES)

F32 = mybir.dt.float32
BF16 = mybir.dt.bfloat16
I32 = mybir.dt.int32


def _preprocess_edges(edge_index):
    """Assign edges (incl. self loops) to the dst-owning core; build per-tile
    round columns (node's r-th edge in its own partition) plus overflow
    one-hot chunk columns.

    Returns (plan, esrcT, edstT):
      plan: list of (R_t, C_t) per tile — shared by all cores.
      esrcT[k] int32 [P, TOTCOL]: gather row ids (pad -> 0).
      edstT[k] f32 [P, TOTCOL]: round cols -> 0.0 real / MASKNEG pad;
                                chunk cols -> dst slot or PAD."""
    src = np.concatenate([edge_index[0], np.arange(N, dtype=np.int64)])
    dst = np.concatenate([edge_index[1], np.arange(N, dtype=np.int64)])
    core = dst // NSH
    dloc = dst - core * NSH
    tile_of = dloc // P
    slot = dloc - tile_of * P
    # gather table layout: SPLITS segments, each rank-major over its row range
    sk = src // NSH
    sr = src % NSH
    split_rows = np.asarray(SPLIT_ROWS, np.int64)
    seg = np.searchsorted(split_rows[1:], sr, side="right")
    r0 = split_rows[seg]
    r1 = split_rows[seg + 1]
    gid = NCORES * r0 + sk * (r1 - r0) + (sr - r0)

    deg = np.zeros((NCORES, NT, P), np.int64)
    np.add.at(deg, (core, tile_of, slot), 1)
    maxdeg_t = deg.max(axis=(0, 2))  # [NT]

    # choose the round cap per tile: chunk columns carry extra one-hot work
    R_ts = np.zeros(NT, np.int64)
    C_ts = np.zeros(NT, np.int64)
    for t in range(NT):
        best = None
        for rcap in range(1, int(maxdeg_t[t]) + 1):
            r = min(int(maxdeg_t[t]), rcap)
            ovf = np.maximum(deg[:, t, :] - rcap, 0).sum(axis=1)
            c = int(np.ceil(ovf / P).max())
            if c > CCAP or r + c > NCAP:
                continue
            cost = r + 1.35 * c
            if best is None or cost < best[0]:
                best = (cost, r, c)
        assert best is not None, f"no feasible plan for tile {t}"
        _, R_ts[t], C_ts[t] = best
    plan = [(int(R_ts[t]), int(C_ts[t])) for t in range(NT)]
    colbase = np.zeros(NT, np.int64)
    colbase[1:] = np.cumsum(R_ts + C_ts)[:-1]
    TOTCOL = int((R_ts + C_ts).sum())

    esrcT = np.zeros((NCORES, P, TOTCOL), np.int32)
    edstT = np.empty((NCORES, P, TOTCOL), np.float32)
    for k in range(NCORES):
        # default fill: rounds masked, chunks PAD
        for t in range(NT):
            b = colbase[t]
            edstT[k, :, b:b + C_ts[t]] = PAD
            edstT[k, :, b + C_ts[t]:b + C_ts[t] + R_ts[t]] = MASKNEG
        m = core == k
        t_k, s_k, g_k = tile_of[m], slot[m], gid[m]
        order = np.argsort(t_k * P + s_k, kind="stable")
        t_k, s_k, g_k = t_k[order], s_k[order], g_k[order]
        node = t_k * P + s_k
        start = np.zeros(NT * P + 1, np.int64)
        np.add.at(start[1:], node, 1)
        start = np.cumsum(start)
        j = np.arange(len(node)) - start[node]  # rank within node
        rmax = R_ts[t_k]
        isr = j < rmax
        # round entries (after the C_t chunk columns)
        rcol = colbase[t_k[isr]] + C_ts[t_k[isr]] + j[isr]
        esrcT[k, s_k[isr], rcol] = g_k[isr].astype(np.int32)
        edstT[k, s_k[isr], rcol] = 0.0
        # overflow entries: sequential position within each tile
        to, so, go = t_k[~isr], s_k[~isr], g_k[~isr]
        oorder = np.argsort(to, kind="stable")
        to, so, go = to[oorder], so[oorder], go[oorder]
        ostart = np.zeros(NT + 1, np.int64)
        np.add.at(ostart[1:], to, 1)
        ostart = np.cumsum(ostart)
        q = np.arange(len(to)) - ostart[to]
        col = colbase[to] + q // P
        row = q % P
        esrcT[k, row, col] = go.astype(np.int32)
        edstT[k, row, col] = so.astype(np.float32)
    return plan, esrcT, edstT


# channel-major permutation: cm index (c*H + h) <- standard (h*C + c)
_PERM_CM = np.array([h * C + c for c in range(C) for h in range(H)], np.int64)


def _ext_weights(W, a_s, a_d, perm_rows):
    """bf16 [P, nk, 528] = [W_cm | W@A_s | W@A_d]; columns chan-major, rows
    permuted chan-major when the layer input is chan-major."""
    K = W.shape[0]
    if perm_rows:
        W = W[_PERM_CM, :]
    Wr = W.reshape(K, H, C)
    ws = np.einsum("fhc,hc->fh", Wr, a_s)
    wd = np.einsum("fhc,hc->fh", Wr, a_d)
    Wx = np.concatenate([W[:, _PERM_CM], ws, wd], axis=1).astype(np.float32)
    nk = K // P
    return np.ascontiguousarray(
        Wx.reshape(nk, P, WEXT).transpose(1, 0, 2)).astype(np_bf16)


def _build_nc(plan):
    TOTCOL = sum(r + c for r, c in plan)
    NCOLMX = max(r + c for r, c in plan)
    CMX = max(c for _, c in plan)
    nc = bacc.Bacc("TRN2", target_bir_lowering=False, debug=False,
                   num_devices=NCORES)

    x_ext = nc.dram_tensor("x0", [NSHP, F_IN], F32, kind="ExternalInput")
    esrc_ext = nc.dram_tensor("esrc", [P, TOTCOL], I32, kind="ExternalInput")
    edst_ext = nc.dram_tensor("edst", [P, TOTCOL], F32, kind="ExternalInput")
    w1_ext = nc.dram_tensor("w1", [P, 1, WEXT], BF16, kind="ExternalInput")
    w2_ext = nc.dram_tensor("w2", [P, 4, WEXT], BF16, kind="ExternalInput")
    w3_ext = nc.dram_tensor("w3", [P, 4, WEXT], BF16, kind="ExternalInput")
    b1_ext = nc.dram_tensor("b1r", [P, HC], BF16, kind="ExternalInput")
    b2_ext = nc.dram_tensor("b2r", [P, HC], BF16, kind="ExternalInput")
    b3_ext = nc.dram_tensor("b3r", [P, C], BF16, kind="ExternalInput")
    pool_ext = nc.dram_tensor("poolidx", [P, NT], F32, kind="ExternalInput")
    invc_ext = nc.dram_tensor("invcnt", [G, 1], F32, kind="ExternalInput")
    fcw_ext = nc.dram_tensor("fcw", [C, NCLS], F32, kind="ExternalInput")
    fcb_ext = nc.dram_tensor("fcbr", [G, NCLS], F32, kind="ExternalInput")
    out_ext = nc.dram_tensor("out", [G, NCLS], F32, kind="ExternalOutput")

    rg = [list(range(NCORES))]

    with tile.TileContext(nc) as tc:
        with (
            tc.tile_pool(name="const", bufs=1) as cpool,
            tc.tile_pool(name="work", bufs=3) as wpool,
            tc.tile_pool(name="gat", bufs=3) as gpool,
            tc.tile_pool(name="ps", bufs=1, space="PSUM") as pspool,
            tc.tile_pool(name="dram", bufs=1, space="DRAM") as dpool,
        ):
            # ---- constants ----
            iota_i = cpool.tile([P, P], I32)
            nc.gpsimd.iota(iota_i[:], pattern=[[1, P]], base=0, channel_multiplier=0)
            iota_f = cpool.tile([P, P], F32)
            nc.vector.tensor_copy(iota_f[:], iota_i[:])
            identf = cpool.tile([P, P], F32)
            make_identity(nc, identf[:])
            identb = cpool.tile([P, P], BF16)
            make_identity(nc, identb[:])

            w1_s = cpool.tile([P, 1, WEXT], BF16)
            nc.sync.dma_start(out=w1_s[:], in_=w1_ext[:])
            w2_s = cpool.tile([P, 4, WEXT], BF16)
            nc.sync.dma_start(out=w2_s[:], in_=w2_ext[:])
            w3_s = cpool.tile([P, 4, WEXT], BF16)
            nc.sync.dma_start(out=w3_s[:], in_=w3_ext[:])
            b1_s = cpool.tile([P, HC], BF16)
            nc.sync.dma_start(out=b1_s[:], in_=b1_ext[:])
            b2_s = cpool.tile([P, HC], BF16)
            nc.sync.dma_start(out=b2_s[:], in_=b2_ext[:])
            b3_s = cpool.tile([P, C], BF16)
            nc.sync.dma_start(out=b3_s[:], in_=b3_ext[:])
            pool_s = cpool.tile([P, NT], F32)
            nc.sync.dma_start(out=pool_s[:], in_=pool_ext[:])
            invc_s = cpool.tile([G, 1], F32)
            nc.sync.dma_start(out=invc_s[:], in_=invc_ext[:])
            fcw_s = cpool.tile([C, NCLS], F32)
            nc.sync.dma_start(out=fcw_s[:], in_=fcw_ext[:])
            fcb_s = cpool.tile([G, NCLS], F32)
            nc.sync.dma_start(out=fcb_s[:], in_=fcb_ext[:])
            es_all = cpool.tile([P, TOTCOL], I32)
            nc.sync.dma_start(out=es_all[:], in_=esrc_ext[:])
            ed_all = cpool.tile([P, TOTCOL], F32)
            nc.sync.dma_start(out=ed_all[:], in_=edst_ext[:])
            # per-node a_dst for current/next layer (f32 adds + bf16 matmul rhs)
            adf_a = cpool.tile([P, NT * H], F32)
            adf_b = cpool.tile([P, NT * H], F32)
            adb_a = cpool.tile([P, NT * H], BF16)
            adb_b = cpool.tile([P, NT * H], BF16)

            # ---- DRAM buffers ----
            hx_local = dpool.tile([NSHP, ROWW], BF16)
            hx_fulls = [
                dpool.tile([NFULL, ROWW], BF16, addr_space="Shared",
                           name=f"hx_full{i}")
                for i in range(3)
            ]
            pool_in = dpool.tile([G, C], F32)
            pool_out = dpool.tile([G, C], F32, addr_space="Shared")

            w_tiles = (w1_s, w2_s, w3_s)
            b_tiles = (b1_s, b2_s, b3_s)
            adf_of = (adf_a, adf_b, adf_a)
            adb_of = (adb_a, adb_b, adb_a)
            split_end = {SPLIT_TILES[i + 1] - 1: i
                         for i in range(len(SPLIT_TILES) - 1)}

            def emit_split_ag(layer, seg):
                r0, r1 = SPLIT_ROWS[seg], SPLIT_ROWS[seg + 1]
                go = NCORES * r0
                nc.gpsimd.collective_compute(
                    "AllGather", mybir.AluOpType.bypass, replica_groups=rg,
                    ins=[hx_local[r0:r1, :]],
                    outs=[hx_fulls[layer][go:go + NCORES * (r1 - r0), :]],
                )

            def stage_a(xt, layer, t):
                """xt: SBUF bf16 [P, K] node-tile features for `layer`; emits
                [h | a_src] -> hx_local rows and a_dst -> adf/adb tiles."""
                K = F_IN if layer == 0 else HC
                nk = K // P
                w_s = w_tiles[layer]
                xT_ps = pspool.tile([P, HC], BF16, tag="trans", bufs=2,
                                    name="xT_ps")
                for j in range(nk):
                    nc.tensor.transpose(out=xT_ps[:, j * P:(j + 1) * P],
                                        in_=xt[:, j * P:(j + 1) * P],
                                        identity=identb[:])
                xT = wpool.tile([P, HC], BF16, tag="xT", name="xT")
                nc.scalar.copy(xT[:, 0:K], xT_ps[:, 0:K])
                h_ps = pspool.tile([P, HC], F32, tag="big", bufs=3, name="h_ps")
                a_ps = pspool.tile([P, 2 * H], F32, tag="small", bufs=3,
                                   name="a_ps")
                for j in range(nk):
                    nc.tensor.matmul(out=h_ps[:], lhsT=xT[:, j * P:(j + 1) * P],
                                     rhs=w_s[:, j, 0:HC],
                                     start=(j == 0), stop=(j == nk - 1))
                    nc.tensor.matmul(out=a_ps[:], lhsT=xT[:, j * P:(j + 1) * P],
                                     rhs=w_s[:, j, HC:WEXT],
                                     start=(j == 0), stop=(j == nk - 1))
                hx_t = wpool.tile([P, ROWW], BF16, tag="hx_t", name="hx_t")
                nc.scalar.copy(hx_t[:, 0:HC], h_ps[:])
                nc.vector.tensor_copy(hx_t[:, HC:ROWW], a_ps[:, 0:H])
                adf_n = adf_of[layer]
                adb_n = adb_of[layer]
                nc.vector.tensor_copy(adf_n[:, t * H:(t + 1) * H], a_ps[:, H:2 * H])
                nc.vector.tensor_copy(adb_n[:, t * H:(t + 1) * H], a_ps[:, H:2 * H])
                nc.sync.dma_start(out=hx_local[t * P:(t + 1) * P, :], in_=hx_t[:])

            # ---- layer-0 stage A (from input features) ----
            for t in range(NT):
                xt_raw = wpool.tile([P, F_IN], F32, tag="xt0", name="xt0")
                nc.sync.dma_start(out=xt_raw[:], in_=x_ext[t * P:(t + 1) * P, :])
                xt_b = wpool.tile([P, F_IN], BF16, tag="xt0b", name="xt0b")
                nc.vector.tensor_copy(xt_b[:], xt_raw[:])
                stage_a(xt_b, 0, t)
                if t in split_end:
                    emit_split_ag(0, split_end[t])

            pool_ps = None
            for layer in range(3):
                hx_full = hx_fulls[layer]
                b_s = b_tiles[layer]
                adf_cur = adf_of[layer]
                adb_cur = adb_of[layer]
                if layer == 2:
                    pool_ps = pspool.tile([G, C], F32, tag="small", bufs=3,
                                          name="pool_ps")
                ch0 = 0
                for t in range(NT):
                    R_t, C_t = plan[t]
                    ncol = R_t + C_t
                    adf = adf_cur[:, t * H:(t + 1) * H]
                    adb = adb_cur[:, t * H:(t + 1) * H]

                    # ---- wide gather: all ncol source rows for this tile ----
                    gt = gpool.tile([P, NCOLMX * ROWW], BF16, tag="gt", name="gt")
                    nc.gpsimd.indirect_dma_start(
                        out=gt[:, 0:ncol * ROWW], out_offset=None,
                        in_=hx_full[:],
                        in_offset=bass.IndirectOffsetOnAxis(
                            ap=es_all[:, ch0:ch0 + ncol], axis=0),
                    )
                    # views of gt: [P, col, 65, H] (chan-major; "chan" 64 = a_src)
                    gtv = gt[:, 0:ncol * ROWW].rearrange(
                        "p (n c h) -> p n c h", n=ncol, c=C + 1, h=H)
                    asrc_v = gtv[:, :, C:C + 1, :]  # [P, ncol, 1, H]

                    logit = wpool.tile([P, NCOLMX * H], F32, tag="logit",
                                       name="logit")
                    lgv = logit[:, 0:ncol * H].rearrange(
                        "p (n h) -> p n h", n=ncol, h=H)

                    # ---- chunk columns: one-hot + a_dst gather ----
                    if C_t > 0:
                        oh = gpool.tile([P, CCAP * P], BF16, tag="oh", name="oh")
                        edv = ed_all[:, ch0:ch0 + C_t].unsqueeze(2).broadcast_to(
                            [P, C_t, P])
                        iov = iota_f[:].unsqueeze(1).broadcast_to([P, C_t, P])
                        ohv = oh[:, 0:C_t * P].rearrange(
                            "p (n q) -> p n q", n=C_t, q=P)
                        nc.vector.tensor_tensor(out=ohv, in0=edv, in1=iov,
                                                op=mybir.AluOpType.is_equal)
                        ohT_ps = pspool.tile([P, CCAP * P], BF16, tag="trans",
                                             bufs=2, name="ohT_ps")
                        for cc in range(C_t):
                            nc.tensor.transpose(
                                out=ohT_ps[:, cc * P:(cc + 1) * P],
                                in_=oh[:, cc * P:(cc + 1) * P],
                                identity=identb[:])
                        ohT = gpool.tile([P, CCAP * P], BF16, tag="ohT",
                                         name="ohT")
                        nc.scalar.copy(ohT[:, 0:C_t * P], ohT_ps[:, 0:C_t * P])
                        adpe_ps = pspool.tile([P, CCAP * H], F32, tag="small",
                                              bufs=3, name="adpe_ps")
                        for cc in range(C_t):
                            nc.tensor.matmul(
                                out=adpe_ps[:, cc * H:(cc + 1) * H],
                                lhsT=ohT[:, cc * P:(cc + 1) * P], rhs=adb,
                                start=True, stop=True)
                        # logit_c = a_src + a_dst[edge]
                        nc.vector.tensor_tensor(
                            out=lgv[:, 0:C_t, :],
                            in0=asrc_v[:, 0:C_t, :, :].squeeze(2),
                            in1=adpe_ps[:, 0:C_t * H].rearrange(
                                "p (n h) -> p n h", n=C_t, h=H),
                            op=mybir.AluOpType.add)

                    # ---- round columns: direct a_dst add + pad mask ----
                    adm = wpool.tile([P, NCOLMX * H], F32, tag="adm", name="adm")
                    admv = adm[:, 0:R_t * H].rearrange(
                        "p (n h) -> p n h", n=R_t, h=H)
                    nc.vector.tensor_tensor(
                        out=admv,
                        in0=adf[:].unsqueeze(1).broadcast_to([P, R_t, H]),
                        in1=ed_all[:, ch0 + C_t:ch0 + ncol].unsqueeze(2)
                            .broadcast_to([P, R_t, H]),
                        op=mybir.AluOpType.add)
                    nc.vector.tensor_tensor(
                        out=lgv[:, C_t:ncol, :],
                        in0=asrc_v[:, C_t:ncol, :, :].squeeze(2),
                        in1=admv, op=mybir.AluOpType.add)

                    # ---- leaky relu + exp ----
                    lr = wpool.tile([P, NCOLMX * H], F32, tag="lr", name="lr")
                    nc.vector.scalar_tensor_tensor(
                        out=lr[:, 0:ncol * H], in0=logit[:, 0:ncol * H],
                        scalar=NEG, in1=logit[:, 0:ncol * H],
                        op0=mybir.AluOpType.mult, op1=mybir.AluOpType.max)
                    ex = wpool.tile([P, NCOLMX * H], F32, tag="ex", name="ex")
                    nc.scalar.activation(ex[:, 0:ncol * H], lr[:, 0:ncol * H],
                                         mybir.ActivationFunctionType.Exp)

                    # ---- softmax denominator, then pre-normalized alpha ----
                    den_r = wpool.tile([P, H], F32, tag="den_r", name="den_r")
                    exrv = ex[:, C_t * H:ncol * H].rearrange(
                        "p (n h) -> p n h", n=R_t, h=H).transpose([0, 2, 1])
                    nc.vector.tensor_reduce(den_r[:], exrv,
                                            axis=mybir.AxisListType.X,
                                            op=mybir.AluOpType.add)
                    den = wpool.tile([P, H], F32, tag="den", name="den")
                    if C_t > 0:
                        exb = wpool.tile([P, CCAP * H], BF16, tag="exb",
                                         name="exb")
                        nc.scalar.copy(exb[:, 0:C_t * H], ex[:, 0:C_t * H])
                        den_ps = pspool.tile([P, H], F32, tag="small", bufs=3,
                                             name="den_ps")
                        for cc in range(C_t):
                            nc.tensor.matmul(out=den_ps[:],
                                             lhsT=oh[:, cc * P:(cc + 1) * P],
                                             rhs=exb[:, cc * H:(cc + 1) * H],
                                             start=(cc == 0),
                                             stop=(cc == C_t - 1))
                        nc.vector.scalar_tensor_tensor(
                            out=den[:], in0=den_r[:], scalar=EPS, in1=den_ps[:],
                            op0=mybir.AluOpType.add, op1=mybir.AluOpType.add)
                    else:
                        nc.vector.tensor_scalar_add(den[:], den_r[:], EPS)
                    rec = wpool.tile([P, H], F32, tag="rec", name="rec")
                    nc.vector.reciprocal(rec[:], den[:])
                    exn = wpool.tile([P, NCOLMX * H], BF16, tag="exn",
                                     name="exn")
                    # round columns: dst == partition, normalize directly
                    nc.vector.tensor_tensor(
                        out=exn[:, C_t * H:ncol * H].rearrange(
                            "p (n h) -> p n h", n=R_t, h=H),
                        in0=ex[:, C_t * H:ncol * H].rearrange(
                            "p (n h) -> p n h", n=R_t, h=H),
                        in1=rec[:].unsqueeze(1).broadcast_to([P, R_t, H]),
                        op=mybir.AluOpType.mult)
                    if C_t > 0:
                        # chunk columns: gather rec[dst] per edge via one-hot
                        rec_b = wpool.tile([P, H], BF16, tag="rec_b",
                                           name="rec_b")
                        nc.vector.tensor_copy(rec_b[:], rec[:])
                        recpe_ps = pspool.tile([P, CCAP * H], F32, tag="small",
                                               bufs=3, name="recpe_ps")
                        for cc in range(C_t):
                            nc.tensor.matmul(
                                out=recpe_ps[:, cc * H:(cc + 1) * H],
                                lhsT=ohT[:, cc * P:(cc + 1) * P], rhs=rec_b[:],
                                start=True, stop=True)
                        nc.vector.tensor_tensor(
                            out=exn[:, 0:C_t * H], in0=ex[:, 0:C_t * H],
                            in1=recpe_ps[:, 0:C_t * H],
                            op=mybir.AluOpType.mult)

                    # ---- messages: h * alpha in one strided op ----
                    msg = gpool.tile([P, NCOLMX * HC], BF16, tag="msg",
                                     name="msg")
                    msgv = msg[:, 0:ncol * HC].rearrange(
                        "p (n c h) -> p n c h", n=ncol, c=C, h=H)
                    exv = exn[:, 0:ncol * H].rearrange(
                        "p (n h) -> p n h", n=ncol, h=H).unsqueeze(2).broadcast_to(
                        [P, ncol, C, H])
                    nc.vector.tensor_tensor(out=msgv, in0=gtv[:, :, 0:C, :],
                                            in1=exv, op=mybir.AluOpType.mult)

                    # ---- weighted scatter-add into PSUM (+ fused bias) ----
                    num_ps = pspool.tile([P, HC], F32, tag="big", bufs=3,
                                         name="num_ps")
                    for cc in range(ncol):
                        lhs = (oh[:, cc * P:(cc + 1) * P] if cc < C_t
                               else identb[:])
                        nc.tensor.matmul(out=num_ps[:], lhsT=lhs,
                                         rhs=msg[:, cc * HC:(cc + 1) * HC],
                                         start=(cc == 0),
                                         stop=(layer == 2 and cc == ncol - 1))
                    if layer < 2:
                        # bias add: identity @ replicated-bias rows
                        nc.tensor.matmul(out=num_ps[:], lhsT=identb[:],
                                         rhs=b_s[:], start=False, stop=True)

                    # ---- finalize ----
                    if layer < 2:
                        xt = wpool.tile([P, HC], BF16, tag="xt", name="xt")
                        nc.scalar.activation(xt[:], num_ps[:],
                                             mybir.ActivationFunctionType.Relu)
                        stage_a(xt, layer + 1, t)
                        if t in split_end:
                            emit_split_ag(layer + 1, split_end[t])
                    else:
                        hm = wpool.tile([P, C], F32, tag="hm", name="hm")
                        nc.vector.tensor_reduce(
                            hm[:],
                            num_ps[:].rearrange("p (c h) -> p c h", c=C, h=H),
                            axis=mybir.AxisListType.X, op=mybir.AluOpType.add)
                        h3f = wpool.tile([P, C], F32, tag="h3f", name="h3f")
                        nc.vector.scalar_tensor_tensor(
                            out=h3f[:], in0=hm[:], scalar=1.0 / H, in1=b3_s[:],
                            op0=mybir.AluOpType.mult, op1=mybir.AluOpType.add)
                        h3 = wpool.tile([P, C], BF16, tag="h3", name="h3")
                        nc.scalar.activation(h3[:], h3f[:],
                                             mybir.ActivationFunctionType.Relu)
                        poh = wpool.tile([P, G], BF16, tag="poh", name="poh")
                        nc.vector.tensor_tensor(
                            out=poh[:],
                            in0=pool_s[:, t:t + 1].to_broadcast([P, G]),
                            in1=iota_f[:, 0:G], op=mybir.AluOpType.is_equal)
                        nc.tensor.matmul(out=pool_ps[:], lhsT=poh[:], rhs=h3[:],
                                         start=(t == 0), stop=(t == NT - 1))
                    ch0 += ncol

            # ---- pool AllReduce + FC + log_softmax ----
            psb = wpool.tile([G, C], F32)
            nc.vector.tensor_copy(psb[:], pool_ps[:])
            nc.sync.dma_start(out=pool_in[:], in_=psb[:])
            nc.gpsimd.collective_compute(
                "AllReduce", mybir.AluOpType.add, replica_groups=rg,
                ins=[pool_in[:]], outs=[pool_out[:]],
            )
            pld = wpool.tile([G, C], F32)
            nc.sync.dma_start(out=pld[:], in_=pool_out[:])
            nc.vector.tensor_scalar_mul(pld[:], pld[:], invc_s[:, 0:1])
            pT_ps = pspool.tile([G, C], F32, tag="small", bufs=3, name="pT_ps")
            nc.tensor.transpose(out=pT_ps[:], in_=pld[:], identity=identf[0:G, 0:C])
            pT = wpool.tile([C, G], F32)
            nc.scalar.copy(pT[:], pT_ps[:])
            z_ps = pspool.tile([G, NCLS], F32, tag="small", bufs=3, name="z_ps")
            nc.tensor.matmul(out=z_ps[:], lhsT=pT[:], rhs=fcw_s[:],
                             start=True, stop=True)
            z = wpool.tile([G, NCLS], F32)
            nc.vector.tensor_add(z[:], z_ps[:], fcb_s[:])
            zm = wpool.tile([G, 1], F32)
            nc.vector.tensor_reduce(zm[:], z[:], axis=mybir.AxisListType.X,
                                    op=mybir.AluOpType.max)
            zs = wpool.tile([G, NCLS], F32)
            nc.vector.tensor_scalar_sub(zs[:], z[:], zm[:, 0:1])
            ze = wpool.tile([G, NCLS], F32)
            nc.scalar.activation(ze[:], zs[:], mybir.ActivationFunctionType.Exp)
            zsum = wpool.tile([G, 1], F32)
            nc.vector.tensor_reduce(zsum[:], ze[:], axis=mybir.AxisListType.X,
                                    op=mybir.AluOpType.add)
            zl = wpool.tile([G, 1], F32)
            nc.scalar.activation(zl[:], zsum[:], mybir.ActivationFunctionType.Ln)
            zo = wpool.tile([G, NCLS], F32)
            nc.vector.tensor_scalar_sub(zo[:], zs[:], zl[:, 0:1])
            nc.sync.dma_start(out=out_ext[:], in_=zo[:])

    nc.compile()
    return nc


def _prepare_inputs(x, edge_index, batch, W1, a1s, a1d, b1, W2, a2s, a2d, b2,
                    W3, a3s, a3d, b3, fcw, fcb):
    plan, esrcT, edstT = _preprocess_edges(np.asarray(edge_index, np.int64))
    x = np.asarray(x, np.float32)
    batch = np.asarray(batch, np.int64)

    shared = {
        "w1": _ext_weights(np.asarray(W1, np.float32), np.asarray(a1s, np.float32),
                           np.asarray(a1d, np.float32), perm_rows=False),
        "w2": _ext_weights(np.asarray(W2, np.float32), np.asarray(a2s, np.float32),
                           np.asarray(a2d, np.float32), perm_rows=True),
        "w3": _ext_weights(np.asarray(W3, np.float32), np.asarray(a3s, np.float32),
                           np.asarray(a3d, np.float32), perm_rows=True),
        "b1r": np.tile(np.asarray(b1, np.float32)[_PERM_CM][None, :],
                       (P, 1)).astype(np_bf16),
        "b2r": np.tile(np.asarray(b2, np.float32)[_PERM_CM][None, :],
                       (P, 1)).astype(np_bf16),
        "b3r": np.tile(np.asarray(b3, np.float32)[None, :], (P, 1)).astype(np_bf16),
        "fcw": np.asarray(fcw, np.float32),
        "fcbr": np.tile(np.asarray(fcb, np.float32)[None, :], (G, 1)),
        "invcnt": (1.0 / np.maximum(
            np.bincount(batch, minlength=G), 1.0)).astype(np.float32)[:, None],
    }

    in_maps = []
    for k in range(NCORES):
        xk = np.zeros((NSHP, F_IN), np.float32)
        xk[:NSH] = x[k * NSH:(k + 1) * NSH]
        pidx = np.full((NSHP,), PAD, np.float32)
        pidx[:NSH] = batch[k * NSH:(k + 1) * NSH]
        poolidx = np.ascontiguousarray(pidx.reshape(NT, P).T)  # [P, NT]
        in_maps.append({
            "x0": xk,
            "esrc": np.ascontiguousarray(esrcT[k]),
            "edst": np.ascontiguousarray(edstT[k]),
            "poolidx": poolidx,
            **shared,
        })
    return plan, in_maps


_CACHE = {}


def _get_nc(plan):
    key = tuple(plan)
    if key not in _CACHE:
        _CACHE[key] = _build_nc(plan)
    return _CACHE[key]


def kernel(x, edge_index, batch, W1, a1s, a1d, b1, W2, a2s, a2d, b2,
           W3, a3s, a3d, b3, fcw, fcb, _trace=False, _results=None):
    plan, in_maps = _prepare_inputs(x, edge_index, batch, W1, a1s, a1d, b1,
                                    W2, a2s, a2d, b2, W3, a3s, a3d, b3, fcw, fcb)
    nc = _get_nc(plan)
    res = run_bass_kernel_spmd(nc, in_maps, core_ids=list(range(NCORES)),
                               trace=_trace)
    if _results is not None:
        _results.append(res)
    return res.results[0]["out"]
